# revision 21
# baseline (speedup 1.0000x reference)
"""Trainium2 Bass kernel for nn_CurveBackBone (VoxFormer-style dual-Hilbert backbone).

kernel(**inputs) takes the FULL unsharded inputs (as produced by setup_inputs())
and returns the full outputs (vox_feats', pts_coors', vox_coors', vox_numbs),
matching the reference.

Sharding: data-parallel over the B=4 scenes x 2 halves = 8 cores.
Core c handles scene b=c//2, ordering-2 half h=c%2:
  - block1 is computed for the FULL scene (the inter-block Hilbert re-ordering
    scatters rows across the whole scene), then rows owned by this core's
    block-2 half are scattered (indirect DMA; bounds-check drops foreign rows)
    into a DRAM buffer in ordering-2 layout;
  - block2 reads that buffer back with a transposing DMA and computes the
    final features for the half-scene.

Host does only index math (Hilbert codes + stable sorts) and layout prep
(gather/transpose/center of raw inputs, block-diagonal weight variants); all
FLOPs and the inter-block permutation run on device.

Device layout: everything feature-major (channels on partitions). Attention is
computed per group-pair: scores S=[k,q] via matmul(lhsT=k_g, rhs=q_g) with the
even group of a pair on PSUM partitions 0-63 and the odd group on 64-127;
softmax uses a per-(pair,query)-max (valid: softmax only needs a per-query
constant) computed by a free-axis DVE max + gpsimd partition-all-reduce;
denominators via a ones-matmul; their reciprocals are broadcast back to all
partitions with a tiny selection matmul. attn@V contracts over tokens with
token-major V produced by streaming h chunks through the PE as lhsT.
"""
import numpy as np

B = 4
N_PER_B = 65536
GS = 64
ORDER = 9
C0, C1, C2 = 64, 128, 128
NCORES = 8

# ---------------------------------------------------------------- host index math

def hilbert_encode_np(coors, order):
    n = 3
    X = [coors[:, i].astype(np.int32) for i in range(n)]
    M = 1 << (order - 1)
    Q = M
    while Q > 1:
        P = Q - 1
        for i in range(n):
            cond = (X[i] & Q) != 0
            if i == 0:
                X[0] = np.where(cond, X[0] ^ P, X[0])
            else:
                t = (X[0] ^ X[i]) & P
                x0 = np.where(cond, X[0] ^ P, X[0] ^ t)
                xi = np.where(cond, X[i], X[i] ^ t)
                X[0], X[i] = x0, xi
        Q >>= 1
    for i in range(1, n):
        X[i] = X[i] ^ X[i - 1]
    t = np.zeros_like(X[0])
    Q = M
    while Q > 1:
        t = np.where((X[n - 1] & Q) != 0, t ^ (Q - 1), t)
        Q >>= 1
    X = [x ^ t for x in X]
    code = np.zeros_like(X[0])
    for b in range(order - 1, -1, -1):
        for i in range(n):
            code = (code << 1) | ((X[i] >> b) & 1)
    return code


def mapping_indices_np(vox_coors):
    vc = np.asarray(vox_coors)
    batch = vc[:, 0]
    code1 = hilbert_encode_np(vc[:, 1:4], ORDER)
    code2 = hilbert_encode_np(vc[:, 3:0:-1], ORDER)
    ind1 = np.lexsort((code1, batch))
    ind2 = np.lexsort((code2, batch))
    inv1 = np.argsort(ind1, kind="stable")
    ind12 = inv1[ind2]
    return ind1, ind2, ind12


def center_groups(p):
    g = np.asarray(p, np.float32).reshape(-1, GS, 3)
    return (g - g.mean(axis=1, keepdims=True)).reshape(-1, 3).astype(np.float32)


def blockdiag2(w):
    k, m = w.shape
    out = np.zeros((2 * k, 2 * m), w.dtype)
    out[:k, :m] = w
    out[k:, m:] = w
    return out

# ---------------------------------------------------------------- device program

_PROG_CACHE = {}


def build_program(SN, debug_taps=False, for_timing=False):
    """Per-core Bass program for scene size SN (block2 half HN = SN//2)."""
    import sys
    if "/opt/trn_rl_repo" not in sys.path:
        sys.path.insert(0, "/opt/trn_rl_repo")
    import concourse.bacc as bacc
    import concourse.tile as tile
    from concourse.tile import add_dep_helper
    from concourse import bass, mybir
    from concourse import bass_isa

    f32 = mybir.dt.float32
    f32r = mybir.dt.float32r
    f16 = mybir.dt.float16
    i16 = mybir.dt.int16
    AF = mybir.ActivationFunctionType
    OP = mybir.AluOpType

    HN = SN // 2
    HALF = HN // 2            # rows per scatter half-buffer
    SC = 512                  # tokens per super-chunk (8 groups)
    NSC1 = SN // SC
    NSC2 = HN // SC
    SGRP = min(8, NSC1)       # super-chunks per scatter group
    NG = NSC1 // SGRP
    NIDX = SGRP * SC          # indices per scatter op
    ZCH = min(4096, HALF)     # zero-init dma chunk (free elems per partition)

    nc = bacc.Bacc("TRN2", target_bir_lowering=False, debug=False)

    # ---- I/O ----
    x1t = nc.dram_tensor("x1t", [C0, SN], f32, kind="ExternalInput")
    pc1t = nc.dram_tensor("pc1t", [4, SN], f32r, kind="ExternalInput")
    pc2t = nc.dram_tensor("pc2t", [4, HN], f32r, kind="ExternalInput")
    sidxa = nc.dram_tensor("sidxa", [128, NG * (NIDX // 16)], i16,
                           kind="ExternalInput")
    sidxb = nc.dram_tensor("sidxb", [128, NG * (NIDX // 16)], i16,
                           kind="ExternalInput")
    w_wpos1 = nc.dram_tensor("w_wpos1", [4, C0], f32r, kind="ExternalInput")
    w_wq1 = nc.dram_tensor("w_wq1", [C0, C0], f16, kind="ExternalInput")
    w_wk1 = nc.dram_tensor("w_wk1", [C0, C0], f16, kind="ExternalInput")
    w_wv1 = nc.dram_tensor("w_wv1", [C0, C0], f16, kind="ExternalInput")
    w_wo1 = nc.dram_tensor("w_wo1", [C0, C0], f32r, kind="ExternalInput")
    w_wout1 = nc.dram_tensor("w_wout1", [C0, C1], f32r, kind="ExternalInput")
    w_wpos2 = nc.dram_tensor("w_wpos2", [4, C1], f32r, kind="ExternalInput")
    w_wq2 = nc.dram_tensor("w_wq2", [C1, C1], f16, kind="ExternalInput")
    w_wk2 = nc.dram_tensor("w_wk2", [C1, C1], f16, kind="ExternalInput")
    w_wv2 = nc.dram_tensor("w_wv2", [C1, C1], f16, kind="ExternalInput")
    w_wo2 = nc.dram_tensor("w_wo2", [C1, C1], f32r, kind="ExternalInput")
    w_wout2 = nc.dram_tensor("w_wout2", [C1, C2], f32r, kind="ExternalInput")
    w_ones64 = nc.dram_tensor("w_ones64", [C0, 1], f16, kind="ExternalInput")
    w_onesb = nc.dram_tensor("w_onesb", [1, 128], f32r, kind="ExternalInput")
    w_ident = nc.dram_tensor("w_ident", [128, 128], f16, kind="ExternalInput")

    bufa = nc.dram_tensor("bufa", [HALF + 1, C1], f16, kind="Internal")
    bufb = nc.dram_tensor("bufb", [HALF + 1, C1], f16, kind="Internal")
    out2t = nc.dram_tensor("out2t", [C2, HN], f32, kind="ExternalOutput")
    if debug_taps:
        dbg_a = nc.dram_tensor("dbg_a", [HALF, C1], f16, kind="ExternalOutput")
        dbg_b = nc.dram_tensor("dbg_b", [HALF, C1], f16, kind="ExternalOutput")

    scatters = []
    scsem = nc.alloc_semaphore("scsem")

    with tile.TileContext(nc) as tc:
        with (
            tc.tile_pool(name="const", bufs=1) as cpool,
            tc.tile_pool(name="sb", bufs=3) as sb,
            tc.tile_pool(name="stg", bufs=2) as stg,
            tc.tile_pool(name="ps", bufs=2, space="PSUM") as ps,
        ):
            def cload(dram, shape, dtype, tag):
                t = cpool.tile(shape, dtype, tag=tag)
                nc.sync.dma_start(t[:], dram[:])
                return t

            wpos1 = cload(w_wpos1, [4, C0], f32r, "wpos1")
            wq1 = cload(w_wq1, [C0, C0], f16, "wq1")
            wk1 = cload(w_wk1, [C0, C0], f16, "wk1")
            wv1 = cload(w_wv1, [C0, C0], f16, "wv1")
            wo1 = cload(w_wo1, [C0, C0], f32r, "wo1")
            wout1 = cload(w_wout1, [C0, C1], f32r, "wout1")
            wpos2 = cload(w_wpos2, [4, C1], f32r, "wpos2")
            wq2 = cload(w_wq2, [C1, C1], f16, "wq2")
            wk2 = cload(w_wk2, [C1, C1], f16, "wk2")
            wv2 = cload(w_wv2, [C1, C1], f16, "wv2")
            wo2 = cload(w_wo2, [C1, C1], f32r, "wo2")
            wout2 = cload(w_wout2, [C1, C2], f32r, "wout2")
            ones64 = cload(w_ones64, [C0, 1], f16, "ones64")
            onesb = cload(w_onesb, [1, 128], f32r, "onesb")
            ident = cload(w_ident, [128, 128], f16, "ident")

            # ---- zero-init the scatter buffers ----
            zt = cpool.tile([128, ZCH], f16, tag="zt")
            nc.gpsimd.memset(zt[:], 0.0)
            inits = []
            for buf in (bufa, bufb):
                bv = buf[0:HALF, :].rearrange("(a b) c -> a (b c)", a=128)
                for j in range(HALF * C1 // (128 * ZCH)):
                    inits.append(nc.sync.dma_start(
                        bv[:, j * ZCH:(j + 1) * ZCH], zt[:]))

            # ================= BLOCK 1 (full scene) ==========================
            stage = None
            for sc in range(NSC1):
                cs = slice(sc * SC, (sc + 1) * SC)
                xs = sb.tile([C0, SC], f32, tag="x1")
                nc.sync.dma_start(xs[:], x1t[:, cs])
                pcs = sb.tile([4, SC], f32r, tag="pc1")
                nc.sync.dma_start(pcs[:], pc1t[:, cs])

                e_ps = ps.tile([C0, SC], f32, tag="psA")
                nc.tensor.matmul(e_ps[:], wpos1[:], pcs[:])
                h = sb.tile([C0, SC], f16, tag="h1")
                nc.vector.tensor_tensor(h[:], e_ps[:], xs[:], op=OP.mult)

                q_ps = ps.tile([C0, SC], f32, tag="psB")
                nc.tensor.matmul(q_ps[:], wq1[:], h[:])
                q = sb.tile([C0, SC], f16, tag="q1")
                nc.any.tensor_copy(q[:], q_ps[:])
                k_ps = ps.tile([C0, SC], f32, tag="psC")
                nc.tensor.matmul(k_ps[:], wk1[:], h[:])
                k = sb.tile([C0, SC], f16, tag="k1")
                nc.any.tensor_copy(k[:], k_ps[:])

                v_ps = ps.tile([C0, SC], f32, tag="psD")
                for g in range(8):
                    gsl = slice(g * 64, (g + 1) * 64)
                    nc.tensor.matmul(v_ps[:, gsl], h[:, gsl], wv1[:])
                v = sb.tile([C0, SC], f16, tag="v1")
                nc.any.tensor_copy(v[:], v_ps[:])

                s_ps = ps.tile([C0, SC], f32, tag="psA")
                for g in range(8):
                    gsl = slice(g * 64, (g + 1) * 64)
                    nc.tensor.matmul(s_ps[:, gsl], k[:, gsl], q[:, gsl])
                s_sb = sb.tile([C0, SC], f32, tag="ssb1")
                nc.scalar.activation(s_sb[:], s_ps[:], AF.Copy)
                mg = sb.tile([C0, SC], f32, tag="mg1")
                nc.gpsimd.partition_all_reduce(
                    mg[:], s_sb[:], channels=C0,
                    reduce_op=bass_isa.ReduceOp.max)
                ss = sb.tile([C0, SC], f16, tag="ss1")
                nc.vector.tensor_tensor(ss[:], s_sb[:], mg[:], op=OP.subtract)
                ee = sb.tile([C0, SC], f16, tag="e1")
                nc.scalar.activation(ee[:], ss[:], AF.Exp)

                d_ps = ps.tile([1, SC], f32, tag="psB")
                nc.tensor.matmul(d_ps[:], ones64[:], ee[:])
                rd = sb.tile([1, SC], f32r, tag="rd1")
                with nc.allow_low_precision(reason="f32r storage"):
                    nc.vector.reciprocal(rd[:], d_ps[:])
                rb_ps = ps.tile([C0, SC], f32, tag="psC")
                nc.tensor.matmul(rb_ps[:], onesb[:, 0:C0], rd[:])
                rb = sb.tile([C0, SC], f16, tag="rb1")
                nc.any.tensor_copy(rb[:], rb_ps[:])

                o_ps = ps.tile([C0, SC], f32, tag="psD")
                for g in range(8):
                    gsl = slice(g * 64, (g + 1) * 64)
                    nc.tensor.matmul(o_ps[:, gsl], v[:, gsl], ee[:, gsl])
                on = sb.tile([C0, SC], f32r, tag="on1")
                nc.vector.tensor_tensor(on[:], o_ps[:], rb[:], op=OP.mult)

                z_ps = ps.tile([C0, SC], f32, tag="psA")
                nc.tensor.matmul(z_ps[:], wo1[:], on[:])
                zr = sb.tile([C0, SC], f32r, tag="zr1")
                nc.scalar.activation(zr[:], z_ps[:], AF.Relu)

                o1_ps = ps.tile([128, SC], f32, tag="psB")
                for j in range(4):
                    nc.tensor.matmul(
                        o1_ps[:, j * 128:(j + 1) * 128],
                        zr[:, j * 128:(j + 1) * 128], wout1[:])

                if sc % SGRP == 0:
                    stage = stg.tile([128, SGRP * SC], f16, tag="stage")
                    gg = sc // SGRP
                    stgate = None
                    if for_timing is False and gg >= 2:
                        # stage slot (bufs=2) was last read by group gg-2's
                        # scatters, whose completion Tile cannot see; gate on
                        # the manual scatter semaphore.
                        stgate = nc.vector.wait_ge(scsem, 32 * (gg - 1))
                        # keep the wait after the scatter it waits for
                        add_dep_helper(stgate.ins,
                                       scatters[2 * (gg - 1) - 1].ins,
                                       reason="wait ordering")
                cp = nc.vector.tensor_copy(
                    stage[:, (sc % SGRP) * SC:(sc % SGRP + 1) * SC], o1_ps[:])
                if stgate is not None:
                    add_dep_helper(cp.ins, stgate.ins, reason="stage reuse")

                if sc % SGRP == SGRP - 1:
                    g = sc // SGRP
                    isl = slice(g * (NIDX // 16), (g + 1) * (NIDX // 16))
                    for sx, buf in ((sidxa, bufa), (sidxb, bufb)):
                        ix = cpool.tile([128, NIDX // 16], i16,
                                        tag=f"ix{g}_{0 if sx is sidxa else 1}")
                        nc.sync.dma_start(ix[:], sx[:, isl])
                        si = nc.gpsimd.dma_scatter_add(
                            out_ap=buf[:],
                            in_ap=stage[:].rearrange("p (c e) -> p c e", e=C1),
                            idxs_ap=ix[:],
                            num_idxs=NIDX,
                            num_idxs_reg=NIDX,
                            elem_size=C1)
                        if not for_timing:
                            si.then_inc(scsem, 16)
                        for ini in inits:
                            add_dep_helper(si.ins, ini.ins, reason="zeroed")
                        scatters.append(si)

            if debug_taps:
                for buf, dbg in ((bufa, dbg_a), (bufb, dbg_b)):
                    nch = HALF * C1 // 128
                    tt = sb.tile([128, nch], f16,
                                 tag="dbg" + ("a" if dbg is dbg_a else "b"))
                    cp = nc.sync.dma_start(
                        tt[:], buf[0:HALF, :].rearrange("(a b) c -> a (b c)", a=128))
                    for s_inst in scatters:
                        add_dep_helper(cp.ins, s_inst.ins, reason="dbg RAW")
                    nc.sync.dma_start(
                        dbg[:].rearrange("(a b) c -> a (b c)", a=128), tt[:])

            # dma_scatter_add carries no auto completion semaphore; wait on
            # the manually attached one before any block-2 read.
            if for_timing:
                scwait = nc.gpsimd.engine_nop()
            else:
                scwait = nc.gpsimd.wait_ge(scsem, 16 * len(scatters))
            for s_inst in scatters:
                add_dep_helper(scwait.ins, s_inst.ins, reason="after scatters")

            # ================= BLOCK 2 (half scene) ==========================
            for sc in range(NSC2):
                boff = sc * SC if sc * SC < HALF else sc * SC - HALF
                buf = bufa if sc * SC < HALF else bufb
                x2m = sb.tile([128, SC], f16, tag="x2m")
                ld = nc.sync.dma_start(
                    x2m[:].rearrange("p (j e) -> p j e", e=C1),
                    buf[boff:boff + SC, :].rearrange("(j p) e -> p j e", p=128))
                add_dep_helper(ld.ins, scwait.ins, reason="x2 RAW")
                x2 = sb.tile([C1, SC], f16, tag="x2")
                for j in range(SC // 128):
                    tp = ps.tile([128, 128], f16, tag="psD")
                    nc.tensor.transpose(
                        tp[:], x2m[:, j * C1:(j + 1) * C1], ident[:])
                    nc.any.tensor_copy(x2[:, j * 128:(j + 1) * 128], tp[:])
                pcs = sb.tile([4, SC], f32r, tag="pc2")
                nc.sync.dma_start(pcs[:], pc2t[:, sc * SC:(sc + 1) * SC])

                e_ps = ps.tile([C1, SC], f32, tag="psA")
                nc.tensor.matmul(e_ps[:], wpos2[:], pcs[:])
                h2 = sb.tile([C1, SC], f16, tag="h2")
                nc.vector.tensor_tensor(h2[:], e_ps[:], x2[:], op=OP.mult)

                q_ps = ps.tile([C1, SC], f32, tag="psB")
                nc.tensor.matmul(q_ps[:], wq2[:], h2[:])
                q2 = sb.tile([C1, SC], f16, tag="q2")
                nc.any.tensor_copy(q2[:], q_ps[:])
                k_ps = ps.tile([C1, SC], f32, tag="psC")
                nc.tensor.matmul(k_ps[:], wk2[:], h2[:])
                k2 = sb.tile([C1, SC], f16, tag="k2")
                nc.any.tensor_copy(k2[:], k_ps[:])

                v2 = sb.tile([C0, SC * 2], f16, tag="v2")
                for half in range(2):
                    vp = ps.tile([C0, SC], f32, tag="psD")
                    for i in range(4):
                        g = half * 4 + i
                        nc.tensor.matmul(
                            vp[:, i * 128:(i + 1) * 128],
                            h2[:, g * 64:(g + 1) * 64], wv2[:])
                    nc.any.tensor_copy(v2[:, half * SC:(half + 1) * SC], vp[:])

                s_ps = ps.tile([C0, SC], f32, tag="psA")
                for g in range(8):
                    gsl = slice(g * 64, (g + 1) * 64)
                    nc.tensor.matmul(s_ps[:, gsl], k2[:, gsl], q2[:, gsl])
                s_sb = sb.tile([C0, SC], f32, tag="ssb2")
                nc.scalar.activation(s_sb[:], s_ps[:], AF.Copy)
                mg = sb.tile([C0, SC], f32, tag="mg2")
                nc.gpsimd.partition_all_reduce(
                    mg[:], s_sb[:], channels=C0,
                    reduce_op=bass_isa.ReduceOp.max)
                ss = sb.tile([C0, SC], f16, tag="ss2")
                nc.vector.tensor_tensor(ss[:], s_sb[:], mg[:], op=OP.subtract)
                ee = sb.tile([C0, SC], f16, tag="e2")
                nc.scalar.activation(ee[:], ss[:], AF.Exp)

                d_ps = ps.tile([1, SC], f32, tag="psB")
                nc.tensor.matmul(d_ps[:], ones64[:], ee[:])
                rd = sb.tile([1, SC], f32r, tag="rd2")
                with nc.allow_low_precision(reason="f32r storage"):
                    nc.vector.reciprocal(rd[:], d_ps[:])
                rb_ps = ps.tile([C1, SC], f32, tag="psC")
                nc.tensor.matmul(rb_ps[:], onesb[:], rd[:])
                rb = sb.tile([C1, SC], f16, tag="rb2")
                nc.any.tensor_copy(rb[:], rb_ps[:])

                o_ps = ps.tile([C1, SC], f32, tag="psA")
                for g in range(8):
                    gsl = slice(g * 64, (g + 1) * 64)
                    nc.tensor.matmul(o_ps[:, gsl],
                                     v2[:, g * 128:(g + 1) * 128],
                                     ee[:, gsl])
                on = sb.tile([C1, SC], f32r, tag="on2")
                nc.vector.tensor_tensor(on[:], o_ps[:], rb[:], op=OP.mult)

                z_ps = ps.tile([C1, SC], f32, tag="psB")
                nc.tensor.matmul(z_ps[:], wo2[:], on[:])
                zr = sb.tile([C1, SC], f32r, tag="zr2")
                nc.scalar.activation(zr[:], z_ps[:], AF.Relu)
                op_ps = ps.tile([C1, SC], f32, tag="psC")
                nc.tensor.matmul(op_ps[:], wout2[:], zr[:])
                ot = sb.tile([C1, SC], f32, tag="ot2")
                nc.any.tensor_copy(ot[:], op_ps[:])
                nc.sync.dma_start(out2t[:, sc * SC:(sc + 1) * SC], ot[:])

    nc.compile()
    return nc


def get_program(SN):
    if SN not in _PROG_CACHE:
        _PROG_CACHE[SN] = build_program(SN)
    return _PROG_CACHE[SN]

# ---------------------------------------------------------------- host prep


def make_weight_maps(inputs):
    f16 = np.float16
    W = {}
    Wpos1 = np.asarray(inputs["Wpos1"], np.float32)
    bpos1 = np.asarray(inputs["bpos1"], np.float32)
    W["w_wpos1"] = np.concatenate([Wpos1, bpos1[None, :]], 0)
    W["w_wq1"] = np.asarray(inputs["Wq1"]).astype(f16)
    W["w_wk1"] = (np.asarray(inputs["Wk1"], np.float32)
                  / np.sqrt(np.float32(C0))).astype(f16)
    W["w_wv1"] = np.asarray(inputs["Wv1"]).astype(f16)
    W["w_wo1"] = np.asarray(inputs["Wo1"], np.float32)
    W["w_wout1"] = np.asarray(inputs["Wout1"], np.float32)
    Wpos2 = np.asarray(inputs["Wpos2"], np.float32)
    bpos2 = np.asarray(inputs["bpos2"], np.float32)
    W["w_wpos2"] = np.concatenate([Wpos2, bpos2[None, :]], 0)
    W["w_wq2"] = np.asarray(inputs["Wq2"]).astype(f16)
    W["w_wk2"] = (np.asarray(inputs["Wk2"], np.float32)
                  / np.sqrt(np.float32(C1))).astype(f16)
    W["w_wv2"] = np.asarray(inputs["Wv2"]).astype(f16)
    W["w_wo2"] = np.asarray(inputs["Wo2"], np.float32)
    W["w_wout2"] = np.asarray(inputs["Wout2"], np.float32)
    W["w_ones64"] = np.ones((C0, 1), f16)
    W["w_onesb"] = np.ones((1, 128), np.float32)
    W["w_ident"] = np.eye(128, dtype=np.float16)
    return W


def make_core_map(feats, pts, ind1, ind2, ind12, h, SN):
    """Per-core data map; ind* are scene-local."""
    HN = SN // 2
    HALF = HN // 2
    NSC1 = SN // 512
    SGRP = min(8, NSC1)
    NIDX = SGRP * 512

    x1 = np.asarray(feats, np.float32)[ind1]
    p1c = center_groups(np.asarray(pts)[ind1])
    m = {"x1t": np.ascontiguousarray(x1.T)}
    m["pc1t"] = np.ascontiguousarray(
        np.concatenate([p1c, np.ones((SN, 1), np.float32)], 1).T)

    j2 = ind2[h * HN:(h + 1) * HN]
    p2c = center_groups(np.asarray(pts)[j2])
    m["pc2t"] = np.ascontiguousarray(
        np.concatenate([p2c, np.ones((HN, 1), np.float32)], 1).T)

    inv12 = np.empty(SN, np.int64)
    inv12[ind12] = np.arange(SN)
    dest = inv12 - h * HN                     # ordering-2 local position
    own = (dest >= 0) & (dest < HN)
    da = np.where(own & (dest < HALF), dest, HALF)
    db = np.where(own & (dest >= HALF), dest - HALF, HALF)
    # wrap the per-op logical index lists into the [128, NIDX/16] tiles
    sa = np.empty((128, (SN // NIDX) * (NIDX // 16)), np.int16)
    sb_ = np.empty_like(sa)
    for g in range(SN // NIDX):
        tok = g * NIDX + np.arange(NIDX)
        wa = da[tok].astype(np.int16).reshape(NIDX // 16, 16).T  # [16, NIDX/16]
        wb = db[tok].astype(np.int16).reshape(NIDX // 16, 16).T
        csl = slice(g * (NIDX // 16), (g + 1) * (NIDX // 16))
        sa[:, csl] = np.tile(wa, (8, 1))
        sb_[:, csl] = np.tile(wb, (8, 1))
    m["sidxa"] = sa
    m["sidxb"] = sb_
    return m


def assemble_core_out(res, HN):
    return np.ascontiguousarray(res["out2t"].T)


def make_in_maps(inputs, SN):
    feats = np.asarray(inputs["vox_feats"], np.float32)
    pts = np.asarray(inputs["pts_coors"], np.float32)
    coors = np.asarray(inputs["vox_coors"], np.int32)
    ind1, ind2, ind12 = mapping_indices_np(coors)
    W = make_weight_maps(inputs)
    in_maps = []
    ncores = (2 * len(feats)) // SN
    for c in range(ncores):
        b, hh = c // 2, c % 2
        sl = slice(b * SN, (b + 1) * SN)
        m = make_core_map(feats[sl], pts[sl], ind1[sl] - b * SN,
                          ind2[sl] - b * SN, ind12[sl] - b * SN, hh, SN)
        m.update(W)
        in_maps.append(m)
    return in_maps, ind1, ind2, ind12

# ---------------------------------------------------------------- entry point


def kernel(**inputs):
    import sys
    if "/opt/trn_rl_repo" not in sys.path:
        sys.path.insert(0, "/opt/trn_rl_repo")
    from concourse import bass_utils

    SN = N_PER_B
    HN = SN // 2
    in_maps, ind1, ind2, ind12 = make_in_maps(inputs, SN)
    nc = get_program(SN)
    res = bass_utils.run_bass_kernel_spmd(nc, in_maps,
                                          core_ids=list(range(NCORES)))

    pts = np.asarray(inputs["pts_coors"], np.float32)
    coors = np.asarray(inputs["vox_coors"], np.int32)
    numbs = np.asarray(inputs["vox_numbs"], np.int32)
    out_feats = np.empty((B * SN, C2), np.float32)
    for c in range(NCORES):
        b, hh = c // 2, c % 2
        out_feats[b * SN + hh * HN: b * SN + (hh + 1) * HN] = \
            assemble_core_out(res.results[c], HN)
    return out_feats, pts[ind2], coors[ind2], numbs
